# revision 59
# baseline (speedup 1.0000x reference)
"""Trainium2 Bass kernel for BatchedCrossAttentionXSMM.

Reference computation (B=1, NQ=NK=2048, A=M=1024, H=16, KD=VD=64):
    q = (q_data @ query_w + query_b) * kd^-0.5      [Q, H, KD]
    k = m_data @ key_w                               [K, H, KD]
    v = m_data @ value_w                             [K, H, VD]
    logits = q k^T + bias                            [H, Q, K]
    w = softmax(logits, axis=-1)
    out = sigmoid(q_data @ gating_w) * (w @ v)       [Q, H, VD]

Sharding: tensor-parallel over heads -- 2 heads per NeuronCore, 8 cores.

Design notes (why it looks the way it does):
 * Streams per core: ACT exp of 2x2048x2048 logits (73us at 1 elem/lane/
   cycle + 352c/instr), PE matmuls (~75us of streaming + per-MM pipeline
   overhead), ~26 MB HBM at ~330 GB/s/core sustained.  Mid-run the PE is
   the pacer (~1.3us per kt-iteration) with ACT right behind (1.15).
 * One flat software-pipelined loop over all 64 (quarter, kt) iterations:
   logits -> exp -> bias-multiply fronts run DELAY=5 iterations ahead of
   the PV accumulation.
 * DMA is THE startup/quarter-0 constraint: three FIFO queues (sync/
   scalar HWDGE + gpsimd SWDGE) share 16 SDMA engines (~330-360 GB/s
   aggregate).  Every queue is emitted in per-queue DEADLINE order (the
   256 B query_b FIRST: in the original order it sat behind 2.25 MB and
   gated the whole front for 13 us).  m/q blocks are half-split across
   both HWDGE queues.  Tiles sharing a pool ring (bias, m) must be
   first-written in consumption order: pool slots bind at first write
   and out-of-order binding deadlocks the slot-WAR chain.  The bias ring
   is 7 deep; 8 measures ~10us WORSE (the 7-ring's WAR pauses
   accidentally load-balance SDMA bandwidth mid-run).
 * HAM (PE clock gate): K=4/8 (1.2 GHz) is the default; ~3.4us of PE
   idle re-throttles and costs ~2x for several us.  Warmup matmuls cover
   the 10-18us icode/DMA ramp, filler matmuls through the pL ring bridge
   (a) the prelude's wait for the second m0/q0 halves, (b) quarter-0's
   DMA-paced micro-stalls (emitted BEFORE the stallable logits: the PE
   queue is in-order), (c) the boundary valleys where the next quarter's
   PV is evacuation-gated for DELAY iterations, and (d) the tail drain.
   With them the HAM stays at 8/8 for one solid ~115us stretch.
 * Projections are emitted in <=4-matmul chunks spread over the
   schedule; K/V of m token-blocks interleave with quarter 0, Q/gating
   for quarter qq+1 run in quarter qq's PE slack (quarter 3's gating
   runs mid-quarter-3 to keep the tail short).  query_b is applied by a
   contraction-1 matmul (bq x ones) to avoid a [128,1] DMA.
 * PSUM (8 banks): 2x 2-bank logits slots + 2x 1-bank projection slots
   + 2x 1-bank PV accumulators.  The PV accumulator rotation forces the
   PSUM evacuation of quarter qq before any PV of qq+1: the evacuation
   is ONE gated tensor_mul per head against persistent [65,512] gate
   tiles whose row 64 is preset to 1.0 (folds sigmoid gate AND keeps
   the softmax denominator), emitted BEFORE the front of that iteration
   so it precedes the et-multiply in the DVE queue.  The rest of the
   fixup (PE transpose, reciprocal, scale-out) is deferred to later,
   lower-pressure iterations -- except quarter 3 (tail).
 * Output is bf16 (host upcasts; ~2e-3 of the 2e-2 budget), head-major
   so the last quarter's two head-halves stream out on both HWDGE
   queues in parallel.

On-device layout: logits are computed transposed ([k, q]) as K Q^T; exp()
lands E^T in SBUF in the layout the PV matmul needs; a [v | 1] stationary
operand produces both weighted values and softmax denominators in one PE
pass.  exp(bias)^T comes bf16 from the host and folds multiplicatively on
the DVE at 2x rate (fp8 inputs blow the 2e-2 max-rel budget: each of
m/key_w/value_w alone costs 1.3-1.8e-2).  The two heads' logits matmuls
are row-group packed (contraction 64 each) so they run concurrently on
the PE.  gpsimd supports ONLY memset/DMA in this walrus build -- its
tensor_* ops fail codegen.
"""
import re
import sys

for _p in ("/opt/trn_rl_repo",):
    if _p not in sys.path:
        sys.path.insert(0, _p)

import ml_dtypes
import numpy as np

import concourse.bass as bass
import concourse.mybir as mybir
import concourse.tile as tile
from concourse.bass_utils import run_bass_kernel_spmd
from concourse.masks import make_identity

BF16 = ml_dtypes.bfloat16
FP8 = ml_dtypes.float8_e4m3fn
WKV_SCALE = 16.0
dt = mybir.dt

NCORES = 8
H_PER_CORE = 2
NQ = NK = 2048
A_DIM = 1024
KD = VD = 64
HC = H_PER_CORE * KD  # 128
SCALE = float(KD) ** -0.5
P = 128
AT = A_DIM // P  # 8 a-subtiles
QT = NQ // P     # 16 token tiles
KT = NK // P
NTB = 4          # 512-token blocks for m/q streaming
NSUB = 4         # bias sub-chunks per q-quarter (4 kt each)


# --- Tile tail-drain patch -------------------------------------------------
# The walrus build in this image caps sem-waits per instruction at 2; Tile's
# kernel-tail drain attaches one wait per live semaphore to a single Drain,
# which fails codegen ("Too many sync wait commands").  Spread the waits over
# a chain of SP nops (1 wait each) before the drain instead.
def _patched_drain_and_barrier(self, tick_clock, wait_clock):
    nc = self.nc
    gc = tick_clock.global_clock
    vals = [int(v) for v in re.findall(r"\d+", repr(gc))]
    alloc = self.sems.allocated()
    waits = []
    for proc, sem in alloc.items():
        v = vals[proc] if proc < len(vals) else 0
        if v > 0:
            mult = 16 if "DMA" in sem.name else 1
            waits.append((sem, v * mult))
    for sem, val in waits:
        nc.sync.nop(nofuse=True).wait_op(sem, val, "sem-ge")
    nc.sync.drain()
    nc.all_engine_barrier()
    popped = nc._tile_sem_poison_stack.pop()
    assert popped is self._sem_poison
    nc.clear_and_free_semaphores(list(self.sems.allocated().values()))
    nc.all_engine_barrier()


tile.TileContext._drain_and_barrier = _patched_drain_and_barrier


# --- BIR wait-splitting pass ----------------------------------------------
# Tile's wait assignment can attach 3+ semaphore waits to a single
# instruction; this walrus build encodes at most 2 wait commands per
# instruction.  Rewrite the serialized BIR: hoist excess waits onto
# preceding EventSemaphore instructions on the same engine.
_MAXW = 1
_orig_to_json_bytes = bass.Bass.to_json_bytes


def _to_json_bytes_split_waits(self):
    import json

    data = json.loads(_orig_to_json_bytes(self))
    ctr = 0
    for fn in data.get("functions", []):
        for bb in fn.get("blocks", []):
            newl = []
            for ins in bb["instructions"]:
                si = ins.get("sync_info")
                if si and si.get("on_wait") and len(si["on_wait"]) > _MAXW:
                    waits = si["on_wait"]
                    extra, keep = waits[:-_MAXW], waits[-_MAXW:]
                    for i in range(0, len(extra), _MAXW):
                        ctr += 1
                        newl.append({
                            "debug": ins.get("debug", 0),
                            "engine": ins["engine"],
                            "ins": [],
                            "outs": [],
                            "name": f"{ins['name']}-wsplit{ctr}",
                            "opcode": "EventSemaphore",
                            "sync_info": {
                                "on_update": [],
                                "on_wait": extra[i:i + _MAXW],
                            },
                        })
                    si["on_wait"] = keep
                newl.append(ins)
            bb["instructions"] = newl
    return json.dumps(data).encode()


bass.Bass.to_json_bytes = _to_json_bytes_split_waits


# --- device program --------------------------------------------------------
def build_nc():
    nc = bass.Bass()
    f32, bf16 = dt.float32, dt.bfloat16
    Exp = mybir.ActivationFunctionType.Exp
    Tanh = mybir.ActivationFunctionType.Tanh

    # m/q host-pre-blocked: row = tb*128 + p, dims [at, 512tok]
    mB_d = nc.dram_tensor("mB", [NTB * P, AT, 512], bf16, kind="ExternalInput")
    qB_d = nc.dram_tensor("qB", [NTB * P, AT, 512], bf16, kind="ExternalInput")
    # exp(bias)^T: row = qq*128 + p, dims [h, kt, 512q]
    ebs_d = nc.dram_tensor("ebs", [4 * P, H_PER_CORE, KT, 512], bf16,
                           kind="ExternalInput")
    wq_d = nc.dram_tensor("wq", [P, AT, HC], bf16, kind="ExternalInput")
    wk_d = nc.dram_tensor("wk", [P, AT, HC], bf16, kind="ExternalInput")
    wv_d = nc.dram_tensor("wv", [P, AT, HC], bf16, kind="ExternalInput")
    wg_d = nc.dram_tensor("wg", [P, AT, HC], bf16, kind="ExternalInput")
    bq_d = nc.dram_tensor("bq", [1, HC], bf16, kind="ExternalInput")
    # output: row = qq*128 + p, dims [h, qt4, vd] (head-major for half-DMAs)
    o_d = nc.dram_tensor("o", [4 * P, H_PER_CORE, 4, VD], bf16,
                         kind="ExternalOutput")

    DELAY = 5  # PV trails the logits/exp/mult front by this many kt

    with tile.TileContext(nc) as tc:
        with (
            tc.tile_pool(name="consts", bufs=1) as consts,
            tc.tile_pool(name="mp", bufs=4) as mp,
            tc.tile_pool(name="qp", bufs=4) as qp,
            tc.tile_pool(name="bp", bufs=7) as bp,
            tc.tile_pool(name="etp", bufs=3) as etp,
            tc.tile_pool(name="gp", bufs=2) as gp,
            tc.tile_pool(name="wsbp", bufs=2) as wsbp,
            tc.tile_pool(name="smallp", bufs=2) as smallp,
            tc.tile_pool(name="outp", bufs=2) as outp,
            tc.tile_pool(name="pL", bufs=2, space="PSUM") as pL,
            tc.tile_pool(name="pS", bufs=2, space="PSUM") as pS,
            tc.tile_pool(name="pW", bufs=2, space="PSUM") as pW,
        ):
            # ---- constants / warmup ----
            id_bf = consts.tile([P, P], bf16, tag="id_bf")
            make_identity(nc, id_bf)
            id_f32 = consts.tile([P, P], f32, tag="id_f32")
            make_identity(nc, id_f32)
            warm_sb = consts.tile([P, 512], bf16, tag="warm_sb")
            nc.vector.memset(warm_sb, 0.0)
            # force the exp ACT-table load off the critical path
            tab_out = consts.tile([P, 16], f32, tag="tab_out")
            nc.scalar.activation(out=tab_out, in_=id_f32[:, 0:16], func=Exp)
            # keep the PE busy until the first blocks land (HAM un-throttle)
            for i in range(34):
                wps_warm = pL.tile([P, 2, 512], f32, tag="pl",
                                   name=f"warm{i}")
                nc.tensor.matmul(wps_warm[:, 0, :], lhsT=id_bf, rhs=warm_sb,
                                 start=True, stop=True)

            # ---- tiles for streamed inputs ----
            mblk = [mp.tile([P, AT, 512], bf16, tag="m", name=f"m{tb}")
                    for tb in range(NTB)]
            qblk = [qp.tile([P, AT, 512], bf16, tag="q", name=f"q{tb}")
                    for tb in range(NTB)]
            # bias ring: uniform 4-kt (1 MB) chunks, 7-deep SBUF ring
            Gq = {0: 4, 1: 4, 2: 4, 3: 4}
            bias_t = {}
            for qq in range(4):
                for s in range(KT // Gq[qq]):
                    bias_t[(qq, s)] = bp.tile(
                        [P, H_PER_CORE, Gq[qq], 512], bf16,
                        tag="bias", name=f"bias{qq}_{s}")
            w_sb = {}
            for name in ("wk", "wv", "wq", "wg"):
                w_sb[name] = consts.tile([P, AT, HC], bf16, tag=f"{name}_sb",
                                         name=f"{name}_sb")
            bq_sb = consts.tile([1, HC], bf16, tag="bq_sb")
            ones_row = consts.tile([1, 512], bf16, tag="ones_row")
            nc.vector.memset(ones_row, 1.0)

            # ---- DMA issue ----
            # Three FIFO queues (sync/scalar HWDGE + gpsimd SWDGE), each
            # ~110-170 GB/s under contention, ~360 aggregate.  Every queue is
            # emitted in DEADLINE order (tiny bq first: a 256 B transfer
            # behind MBs of FIFO traffic stalled the whole front for 13 us).
            # HWDGE (sy+sc) calls are interleaved in expected-completion
            # order so the 8 shared sem lanes stay monotonic per lane.
            def mdma(eng, tb, part=None, w=4):
                ats = slice(None) if part is None else slice(part * w,
                                                            part * w + w)
                eng.dma_start(out=mblk[tb][:, ats, :],
                              in_=mB_d[tb * P:(tb + 1) * P, ats, :])

            def qdma(eng, tb, part=None, w=4):
                ats = slice(None) if part is None else slice(part * w,
                                                            part * w + w)
                eng.dma_start(out=qblk[tb][:, ats, :],
                              in_=qB_d[tb * P:(tb + 1) * P, ats, :])

            def bdma(eng, qq, s):
                G = Gq[qq]
                eng.dma_start(out=bias_t[(qq, s)],
                              in_=ebs_d[qq * P:(qq + 1) * P, :,
                                        s * G:(s + 1) * G, :])

            def wdma(eng, name, d):
                eng.dma_start(out=w_sb[name], in_=d[:, :, :])

            sy, sc, g = nc.sync, nc.scalar, nc.gpsimd
            # Global emission order: per-queue sequences are deadline-
            # ordered, AND tiles sharing a pool ring (bias, m) are first-
            # written in consumption order (pool slots bind at first write;
            # out-of-order binding deadlocks the slot WAR chain).  The m
            # blocks are half-split across both HWDGE queues so quarter-0's
            # K/V projections never wait behind a full 1 MB transfer.
            sy.dma_start(out=bq_sb, in_=bq_d[:, :])
            wdma(sc, "wq", wq_d)
            wdma(sy, "wk", wk_d)
            wdma(g, "wv", wv_d)
            qdma(sc, 0, 0)
            mdma(sy, 0, 0)
            qdma(sc, 0, 1)
            mdma(sy, 0, 1)
            bdma(g, 0, 0)
            mdma(sc, 1, 0)
            mdma(sy, 1, 1)
            bdma(g, 0, 1)
            mdma(sc, 2, 0)
            mdma(sy, 2, 1)
            mdma(sc, 3, 0)
            mdma(g, 3, 1)
            bdma(sy, 0, 2)
            bdma(g, 0, 3)
            qdma(sc, 1)
            wdma(g, "wg", wg_d)
            bdma(sy, 1, 0)
            bdma(sc, 1, 1)
            bdma(g, 1, 2)
            bdma(sy, 1, 3)
            qdma(g, 2)
            bdma(sc, 2, 0)
            bdma(sy, 2, 1)
            bdma(g, 2, 2)
            bdma(sc, 2, 3)
            qdma(sc, 3)
            bdma(sy, 3, 0)
            bdma(sc, 3, 1)
            bdma(g, 3, 2)
            bdma(sy, 3, 3)

            # ---- persistent SBUF ----
            kT2 = consts.tile([HC, NK], bf16, tag="kT2")
            qT2 = consts.tile([HC, NQ], bf16, tag="qT2")
            vT2 = consts.tile([HC, NK], bf16, tag="vT2")
            v_sb = consts.tile([P, H_PER_CORE, KT, VD + 1], bf16, tag="v_sb")
            nc.vector.memset(v_sb, 1.0)

            # ---- projection helpers (emitted in <=4-MM chunks) ----
            held = {}

            def kv_mms(w, tb, rng, name):
                ps = held.get(name)
                if ps is None:
                    ps = pS.tile([P, 512], f32, tag="ps1", name=name)
                    held[name] = ps
                for at in rng:
                    nc.tensor.matmul(ps, lhsT=w_sb[w][:, at, :],
                                     rhs=mblk[tb][:, at, :] if w in ("wk", "wv")
                                     else qblk[tb][:, at, :],
                                     start=(at == 0),
                                     stop=(at == AT - 1 and w != "wq"))
                if w == "wq" and rng[-1] == AT - 1:
                    # + query_b via a contraction-1 matmul (bq ⊗ ones)
                    nc.tensor.matmul(ps, lhsT=bq_sb, rhs=ones_row,
                                     start=False, stop=True)
                return ps

            def k_copy(tb):
                tbs = slice(tb * 512, (tb + 1) * 512)
                nc.vector.tensor_copy(out=kT2[:, tbs], in_=held.pop(f"psK{tb}"))

            def v_copy(tb):
                tbs = slice(tb * 512, (tb + 1) * 512)
                nc.vector.tensor_copy(out=vT2[:, tbs], in_=held.pop(f"psV{tb}"))

            def v_tp(kt):
                tps = pS.tile([P, P], f32, tag="ps1", name=f"vtp{kt}")
                nc.tensor.matmul(tps, lhsT=vT2[:, kt * P:(kt + 1) * P],
                                 rhs=id_bf, start=True, stop=True)
                nc.vector.tensor_copy(
                    out=v_sb[:, :, kt, 0:VD],
                    in_=tps.rearrange("p (h c) -> p h c", h=H_PER_CORE),
                )

            def q_affine(qq):
                qs = slice(qq * 512, (qq + 1) * 512)
                nc.vector.tensor_scalar_mul(
                    out=qT2[:, qs], in0=held.pop(f"psQ{qq}"), scalar1=SCALE)

            # persistent [65,512] gate tiles, row 64 preset to 1.0: the PSUM
            # evacuation then folds sigmoid-gate AND keeps the denominator in
            # ONE tensor_mul per head (ping-pong by quarter parity)
            gab = [consts.tile([VD + 1, 512], f32, tag=f"gab{i}",
                               name=f"gab{i}") for i in range(4)]
            for i in range(4):
                nc.vector.memset(gab[i], 1.0)

            def g_finish(qq):
                psG = held.pop(f"psG{qq}")
                gth = gp.tile([P, 512], f32, tag="gth", name=f"gth{qq}")
                nc.scalar.activation(out=gth, in_=psG, func=Tanh, scale=0.5)
                for h in range(H_PER_CORE):
                    hs = slice(h * KD, (h + 1) * KD)
                    nc.vector.tensor_scalar(
                        out=gab[2 * (qq % 2) + h][0:VD, :], in0=gth[hs, :],
                        scalar1=0.5, scalar2=0.5,
                        op0=mybir.AluOpType.mult, op1=mybir.AluOpType.add,
                    )

            wps_q = {}

            def fixup_p0(qq):
                # both wsb evacuations must be emitted before any PV of
                # quarter qq+1 (the pw slot rotation reuses these banks).
                # One gated tensor_mul per head is the whole evacuation.
                out_t = outp.tile([P, H_PER_CORE, 4, VD], bf16, tag="out",
                                  name=f"out{qq}")
                held[f"out{qq}"] = out_t
                for h in range(H_PER_CORE):
                    wsb = wsbp.tile([P, 512], f32, tag="wsb",
                                    name=f"wsb{qq}_{h}")
                    nc.vector.tensor_mul(out=wsb[0:VD + 1, :],
                                         in0=wps_q[qq][h][0:VD + 1, :],
                                         in1=gab[2 * (qq % 2) + h])
                    held[f"wsb{qq}_{h}"] = wsb

            def fixup_h(qq, h):
                wsb = held.pop(f"wsb{qq}_{h}")
                out_t = held[f"out{qq}"]
                tp4 = pS.tile([P, 4, VD + 1], f32, tag="ps1",
                              name=f"tp4_{qq}_{h}")
                for qb in range(4):
                    nc.tensor.matmul(
                        tp4[:, qb, :],
                        lhsT=wsb[0:VD + 1, qb * P:(qb + 1) * P],
                        rhs=id_f32[0:VD + 1, 0:VD + 1],
                        is_transpose=True, start=True, stop=True,
                    )
                rec4 = smallp.tile([P, 4], f32, tag="rec", name=f"rec{qq}_{h}")
                nc.vector.reciprocal(out=rec4, in_=tp4[:, :, VD])
                for qb in range(4):
                    nc.vector.tensor_scalar_mul(
                        out=out_t[:, h, qb, :], in0=tp4[:, qb, 0:VD],
                        scalar1=rec4[:, qb:qb + 1])

            def out_dma(qq, h):
                out_t = held[f"out{qq}"]
                eng = nc.sync if h == 0 else nc.scalar
                eng.dma_start(
                    out=o_d[qq * P:(qq + 1) * P, h, :, :],
                    in_=out_t[:, h, :, :])

            def fixup_p1(qq):
                fixup_h(qq, 1)
                if qq == 3:
                    out_dma(qq, 1)
                    held.pop(f"out{qq}")
                else:
                    out_t = held.pop(f"out{qq}")
                    nc.gpsimd.dma_start(out=o_d[qq * P:(qq + 1) * P, :, :, :],
                                        in_=out_t)

            def fixup_p0_tail(qq):
                fixup_h(qq, 0)
                if qq == 3:
                    out_dma(qq, 0)

            # ---- injection schedule (u = global kt index of the front) ----
            # presched runs BEFORE the front of iteration u (evacuations must
            # precede the et-multiply in the DVE queue, or PV waits longer)
            sched = {}
            presched = {}

            def add(u, fn):
                sched.setdefault(u, []).append(fn)

            def pre_add(u, fn):
                presched.setdefault(u, []).append(fn)

            H4 = AT // 2  # 4
            for tb in range(1, NTB):
                u0 = 4 * (tb - 1) + 2
                add(u0 + 0, lambda tb=tb: kv_mms("wk", tb, range(0, H4),
                                                 f"psK{tb}"))
                add(u0 + 1, lambda tb=tb: (kv_mms("wk", tb, range(H4, AT),
                                                  f"psK{tb}"), k_copy(tb)))
                add(u0 + 2, lambda tb=tb: kv_mms("wv", tb, range(0, H4),
                                                 f"psV{tb}"))
                add(u0 + 3, lambda tb=tb: (kv_mms("wv", tb, range(H4, AT),
                                                  f"psV{tb}"), v_copy(tb)))
                add(u0 + 4, lambda tb=tb: [v_tp(tb * 4 + j) for j in range(2)])
                add(u0 + 5, lambda tb=tb: [v_tp(tb * 4 + j) for j in range(2, 4)])
            # tb0's V-side: keep u=0..1 clear so nothing competes with the
            # first logits/exp on the PE FIFO
            add(1, lambda: kv_mms("wv", 0, range(0, H4), "psV0"))
            add(2, lambda: (kv_mms("wv", 0, range(H4, AT), "psV0"), v_copy(0)))
            add(3, lambda: [v_tp(j) for j in range(2)])
            add(4, lambda: [v_tp(j) for j in range(2, 4)])
            # Q projection for quarter qq+1 late in quarter qq
            for qq in range(3):
                add(qq * 16 + 13, lambda q=qq + 1: kv_mms("wq", q, range(0, H4),
                                                          f"psQ{q}"))
                add(qq * 16 + 14, lambda q=qq + 1: (
                    kv_mms("wq", q, range(H4, AT), f"psQ{q}"), q_affine(q)))
            # gating projection for quarter qq early in quarter qq+1
            # (quarter 3's runs inside its own quarter so the tail is short)
            for qq in range(4):
                u_g = qq * 16 + 17 if qq < 3 else 3 * 16 + 9
                for part in range(4):
                    add(u_g + part,
                        lambda q=qq, p=part: kv_mms("wg", q,
                                                    range(2 * p, 2 * p + 2),
                                                    f"psG{q}"))
                add(u_g + 3, lambda q=qq: g_finish(q))
            # dummy pL-ring matmuls: keep the PE above HAM's idle threshold
            # through the boundary valleys (PV of the next quarter is evac-
            # gated for ~DELAY iterations) and through the tail drain
            _fill_n = [0]

            def pe_fill(n=2):
                for _ in range(n):
                    _fill_n[0] += 1
                    wt = pL.tile([P, 2, 512], f32, tag="pl",
                                 name=f"fill{_fill_n[0]}")
                    nc.tensor.matmul(wt[:, 0, :], lhsT=id_bf, rhs=warm_sb,
                                     start=True, stop=True)

            for qq in range(3):
                for du in (19, 20, 21):
                    add(qq * 16 + du, pe_fill)
            for u_t in (64, 65, 66, 67):
                add(u_t, pe_fill)
            # quarter 0 runs DMA-paced: keep the PE warm through the
            # m/bias-arrival stalls.  These must run BEFORE the (possibly
            # stalled) logits of the iteration -- the PE queue is in-order.
            for u_t in range(3, 11):
                pre_add(u_t, pe_fill)

            # fixups: the bank-releasing evacuation at +21; the rest spread
            # over later, lower-pressure iterations (except quarter 3: tail)
            for qq in range(4):
                pre_add(qq * 16 + 21, lambda q=qq: fixup_p0(q))
                if qq < 3:
                    add(qq * 16 + 24, lambda q=qq: fixup_p0_tail(q))
                    add(qq * 16 + 26, lambda q=qq: fixup_p1(q))
                else:
                    add(qq * 16 + 21, lambda q=qq: fixup_p0_tail(q))
                    add(qq * 16 + 22, lambda q=qq: fixup_p1(q))

            # ---- prelude: K projection of tb0 + Q projection of quarter 0
            # (first halves as soon as m0a/q0a land; fillers bridge the
            # ~5 us wait for the second halves so HAM never re-throttles)
            kv_mms("wk", 0, range(0, H4), "psK0")
            kv_mms("wq", 0, range(0, H4), "psQ0")
            pe_fill(14)
            kv_mms("wk", 0, range(H4, AT), "psK0")
            k_copy(0)
            kv_mms("wq", 0, range(H4, AT), "psQ0")
            q_affine(0)

            # ---- main software-pipelined loop ----
            ets = {}
            for u in range(4 * KT + DELAY + 2):
                for fn in presched.get(u, []):
                    fn()
                if u < 4 * KT:
                    qq, kt = divmod(u, KT)
                    qs = slice(qq * 512, (qq + 1) * 512)
                    if kt == 0:
                        wps_q[qq] = [
                            pW.tile([P, 512], f32, tag="pw",
                                    name=f"wps{qq}_{h}")
                            for h in range(H_PER_CORE)]
                    ks = slice(kt * P, (kt + 1) * P)
                    lpp = pL.tile([P, H_PER_CORE, 512], f32, tag="pl",
                                  name=f"lpp{qq}_{kt}")
                    for h in range(H_PER_CORE):
                        hs = slice(h * KD, (h + 1) * KD)
                        nc.tensor.matmul(
                            lpp[:, h, :],
                            lhsT=kT2[hs, ks], rhs=qT2[hs, qs],
                            start=True, stop=True,
                        )
                    etr = etp.tile([P, H_PER_CORE, 512], bf16, tag="etr",
                                   name=f"etr{qq}_{kt}")
                    nc.scalar.activation(out=etr, in_=lpp, func=Exp)
                    et = etp.tile([P, H_PER_CORE, 512], bf16, tag="et",
                                  bufs=DELAY + 3, name=f"et{qq}_{kt}")
                    G = Gq[qq]
                    nc.vector.tensor_mul(
                        out=et, in0=etr,
                        in1=bias_t[(qq, kt // G)][:, :, kt % G, :])
                    ets[u] = et
                for fn in sched.get(u, []):
                    fn()
                b = u - DELAY
                if 0 <= b < 4 * KT:
                    bqq, bkt = divmod(b, KT)
                    for h in range(H_PER_CORE):
                        nc.tensor.matmul(
                            wps_q[bqq][h][0:VD + 1, :],
                            lhsT=v_sb[:, h, bkt, :],
                            rhs=ets[b][:, h, :],
                            start=(bkt == 0), stop=(bkt == KT - 1),
                        )
                    del ets[b]

    return nc


_NC = None


def _get_nc():
    global _NC
    if _NC is None:
        _NC = build_nc()
    return _NC


# --- host side -------------------------------------------------------------
def prepare_in_maps(q_data, m_data, batched_bias, query_w, query_b, key_w,
                    value_w, gating_w):
    q = np.asarray(q_data, np.float32)[0]          # [NQ, A]
    m = np.asarray(m_data, np.float32)[0]          # [NK, A]
    bias = np.asarray(batched_bias, np.float32)[0]  # [H, NQ, NK]
    bq = np.asarray(query_b, np.float32)[0]        # [H, KD]

    # [A, N] -> token blocks [tb*128+p, at, 512]
    def blockify(x):
        xT = x.T  # [A, N]
        b = xT.reshape(AT, P, NTB, 512).transpose(2, 1, 0, 3)
        return np.ascontiguousarray(b.reshape(NTB * P, AT, 512)).astype(BF16)

    qB = blockify(q)
    mB = blockify(m)

    def wslice(w, c):
        w = np.asarray(w, np.float32)
        ws = w[:, 2 * c:2 * c + 2, :].reshape(A_DIM, HC)
        ws = ws.reshape(AT, P, HC).transpose(1, 0, 2)
        return np.ascontiguousarray(ws).astype(BF16)

    in_maps = []
    for c in range(NCORES):
        # exp(bias)^T  [h, k, q] -> sub-chunks [(qq*4+g)*128+p, h, j, 512]
        ebT = np.exp(bias[2 * c:2 * c + 2].transpose(0, 2, 1))
        # dims: [h, kt, p, qq, q'] -> [qq, p, h, kt, q']
        eb = ebT.reshape(H_PER_CORE, KT, P, 4, 512).transpose(3, 2, 0, 1, 4)
        ebs = np.ascontiguousarray(
            eb.reshape(4 * P, H_PER_CORE, KT, 512)).astype(BF16)
        in_maps.append({
            "qB": qB,
            "mB": mB,
            "ebs": ebs,
            "wq": wslice(query_w, c),
            "wk": wslice(key_w, c),
            "wv": wslice(value_w, c),
            "wg": wslice(gating_w, c),
            "bq": np.ascontiguousarray(bq[2 * c:2 * c + 2].reshape(1, HC)).astype(BF16),
        })
    return in_maps


def gather_out(results):
    # o: [qq*128+p, h, qt4, vd] -> [NQ, H_PER_CORE, VD]
    parts = []
    for r in results:
        o = np.asarray(r["o"]).reshape(4, P, H_PER_CORE, 4, VD)
        o = o.transpose(0, 3, 1, 2, 4).reshape(NQ, H_PER_CORE, VD)
        parts.append(o)
    return np.concatenate(parts, axis=1)[None].astype(np.float32)


def kernel(**inputs):
    in_maps = prepare_in_maps(**inputs)
    res = run_bass_kernel_spmd(_get_nc(), in_maps, core_ids=list(range(NCORES)))
    return gather_out(res.results)

